# revision 20
# baseline (speedup 1.0000x reference)
"""GroupedQueryAttention TRN2 Bass kernel, sharded over 8 NeuronCores.

Problem (hardcoded): B=2, T=2048, D=4096, 32 Q heads x 128, 8 KV groups x 128,
RoPE (base 5e5), causal, out = ctx @ Wo.

Sharding: core g owns Q heads 4g..4g+3 (Wq columns 512g:512g+512), KV group g
(Wk/Wv columns 128g:128g+128), and Wo rows 512g:512g+512 (row-parallel).
Each core computes a full-shape partial output; host sums the 8 partials.

All matmuls run in bf16 (host-converted inputs), accumulation in f32 PSUM.
Q and attention context stay resident in SBUF (no DRAM roundtrips).
One PSUM pool spans all three phases (no pool-transition barriers); softmax
normalization runs off the PE (fast DVE reciprocal + GPSIMD partition
broadcast); weight/x loads issue from the GPSIMD sequencer with merged
access patterns.
"""
import sys
import numpy as np

for _p in ("/opt/trn_rl_repo", "/root/.axon_site", "/root/.axon_site/_ro/trn_rl_repo"):
    if _p not in sys.path:
        sys.path.append(_p)

from contextlib import ExitStack

import ml_dtypes

import concourse.bass as bass
import concourse.tile as tile
from concourse import bacc, mybir
from concourse.bass_utils import run_bass_kernel_spmd
from concourse.masks import make_identity

B, T, D = 2, 2048, 4096
NH, NKV, DH = 32, 8, 128
HPC = NH // 8          # 4 q heads per core
FPC = HPC * DH         # 512 q features per core
ROPE_BASE = 500000.0
NT = B * T             # 4096 tokens
KC = D // 128          # 32 contraction chunks
SLAB = 4
NSLAB = KC // SLAB
f32 = mybir.dt.float32
bf16 = mybir.dt.bfloat16
EXP_SCALE = 1.0 / float(np.sqrt(DH))
BF = ml_dtypes.bfloat16

_NC_CACHE = {}


def _build_program():
    nc = bacc.Bacc("TRN2", target_bir_lowering=False, debug=False)

    xT = nc.dram_tensor("xT", [D, NT], bf16, kind="ExternalInput").ap()
    wq = nc.dram_tensor("wq", [D, FPC], bf16, kind="ExternalInput").ap()
    wk = nc.dram_tensor("wk", [D, DH], bf16, kind="ExternalInput").ap()
    wv = nc.dram_tensor("wv", [D, DH], bf16, kind="ExternalInput").ap()
    wo = nc.dram_tensor("wo", [FPC, D], bf16, kind="ExternalInput").ap()
    ropeA = nc.dram_tensor("ropeA", [128, T], f32, kind="ExternalInput").ap()
    ropeB = nc.dram_tensor("ropeB", [128, T], f32, kind="ExternalInput").ap()
    masks = nc.dram_tensor("masks", [512, 512], bf16, kind="ExternalInput").ap()
    outp = nc.dram_tensor("outp", [NT, D], bf16, kind="ExternalOutput").ap()

    with tile.TileContext(nc) as tc, ExitStack() as s0:
        top = s0.enter_context(tc.tile_pool(name="top", bufs=1))
        KTb = [top.tile([128, T], bf16, tag=f"KT{i}", name=f"KT{i}") for i in range(B)]
        Vb = [top.tile([128, T], bf16, tag=f"V{i}", name=f"V{i}") for i in range(B)]
        Qres = top.tile([128, HPC * NT], bf16, tag="qres")
        ctxr = top.tile([128, HPC * NT], bf16, tag="ctxr")
        msk = top.tile([128, 4 * 512], bf16, tag="msk")
        ident = top.tile([128, 128], bf16, tag="ident")
        ident_f = top.tile([128, 128], f32, tag="ident_f")
        ones_f = top.tile([128, 128], f32, tag="ones_f")
        ones = top.tile([128, 128], bf16, tag="ones")
        # phase-B SBUF tiles live at top level so B's pipeline never waits on
        # phase-A pool release
        bp = s0.enter_context(tc.tile_pool(name="bp", bufs=1))
        # single PSUM pool for all phases: two ping-pong slot groups
        psp = s0.enter_context(tc.tile_pool(name="psp", bufs=1, space="PSUM"))

        def T1(shape, name, dt=f32):
            return psp.tile(shape, dt, tag="T1", bufs=2, name=name)

        def T2(shape, name, dt=f32):
            return psp.tile(shape, dt, tag="T2", bufs=2, name=name)

        # phase-B item stream (defined early: the tail of phase A primes it)
        items = []
        for b_ in range(B):
            for h_ in range(HPC):
                for qt_ in range(4):
                    plist = [2 * qt_, 2 * qt_ + 1] + list(range(0, 2 * qt_))
                    for idx_, p_ in enumerate(plist):
                        items.append((b_, h_, qt_, p_, idx_, idx_ == len(plist) - 1))
        LOOK = 3
        se_q = []

        def emit_S(it):
            b, h, qt, p, idx, last = it
            st = T1([128, 1024], "st")
            qmv = Qres[:, h * NT + b * T + qt * 512: h * NT + b * T + (qt + 1) * 512]
            for half in range(2):
                kt = 2 * p + half
                nc.tensor.matmul(st[:, half * 512:(half + 1) * 512],
                                 KTb[b][:, kt * 128:(kt + 1) * 128],
                                 qmv, start=True, stop=True)
            se = bp.tile([128, 1024], bf16, tag="se", bufs=5, name="se")
            nc.scalar.activation(se[:], st[:],
                                 mybir.ActivationFunctionType.Exp,
                                 scale=EXP_SCALE)
            if p >= 2 * qt:
                r = (p - 2 * qt) * 2
                nc.vector.tensor_mul(se[:], se[:], msk[:, r * 512:(r + 2) * 512])
            # pair-sum on DVE so one sum-MM per pair suffices
            ss = bp.tile([128, 512], bf16, tag="ss", bufs=5, name="ss")
            nc.vector.tensor_add(ss[:], se[:, 0:512], se[:, 512:1024])
            return se, ss

        # ---------------- Phase A: projections + RoPE -----------------
        with ExitStack() as sa:
            wp = sa.enter_context(tc.tile_pool(name="wts", bufs=1))
            wq_sb = wp.tile([128, KC * FPC], bf16, tag="wq")
            wk_sb = wp.tile([128, KC * DH], bf16, tag="wk")
            wv_sb = wp.tile([128, KC * DH], bf16, tag="wv")
            tabA = wp.tile([128, T], f32, tag="tabA")
            tabB = wp.tile([128, T], f32, tag="tabB")

            # x/weight loads issue from the GPSIMD sequencer (cheap SWDGE
            # dispatch) with slab-merged access patterns (few large starts)
            def load_w_slab(s):
                r0 = s * 512
                if s == 0:
                    # fine-grained wq for first-matmul latency
                    nc.gpsimd.dma_start(wq_sb[:, 0:256], wq[0:128, 0:256])
                    nc.gpsimd.dma_start(wq_sb[:, 256:512], wq[0:128, 256:512])
                    for k in range(1, SLAB):
                        nc.gpsimd.dma_start(wq_sb[:, k * FPC:(k + 1) * FPC],
                                            wq[k * 128:(k + 1) * 128, :])
                    nc.gpsimd.dma_start(
                        wk_sb[:, 0:SLAB * DH].rearrange("p (k f) -> p k f", k=4),
                        wk[0:512, :].rearrange("(k p) f -> p k f", k=4))
                    nc.gpsimd.dma_start(
                        wv_sb[:, 0:SLAB * DH].rearrange("p (k f) -> p k f", k=4),
                        wv[0:512, :].rearrange("(k p) f -> p k f", k=4))
                    return
                for half in range(2):
                    src = wq[r0 + half * 256:r0 + (half + 1) * 256, :].rearrange(
                        "(k p) f -> p k f", k=2)
                    dst = wq_sb[:, (s * SLAB + half * 2) * FPC:
                                (s * SLAB + (half + 1) * 2) * FPC].rearrange(
                        "p (k f) -> p k f", k=2)
                    nc.gpsimd.dma_start(dst, src)
                nc.gpsimd.dma_start(
                    wk_sb[:, s * SLAB * DH:(s + 1) * SLAB * DH].rearrange(
                        "p (k f) -> p k f", k=4),
                    wk[r0:r0 + 512, :].rearrange("(k p) f -> p k f", k=4))
                nc.gpsimd.dma_start(
                    wv_sb[:, s * SLAB * DH:(s + 1) * SLAB * DH].rearrange(
                        "p (k f) -> p k f", k=4),
                    wv[r0:r0 + 512, :].rearrange("(k p) f -> p k f", k=4))

            xsp = sa.enter_context(tc.tile_pool(name="xs", bufs=1))
            evp = sa.enter_context(tc.tile_pool(name="ev", bufs=1))

            def stationary(m, k):
                if m < HPC:
                    return wq_sb[:, k * FPC + m * 128: k * FPC + (m + 1) * 128]
                if m == HPC:
                    return wk_sb[:, k * DH:(k + 1) * DH]
                return wv_sb[:, k * DH:(k + 1) * DH]

            xsl_tiles = {}

            def prefetch_x(n, s):
                if n >= NT // 512:
                    return
                t = xsp.tile([128, SLAB * 512], bf16, tag="xs", bufs=3, name="xsl")
                if n == 0 and s < 2:
                    for j in range(SLAB):
                        k = s * SLAB + j
                        nc.sync.dma_start(t[:, j * 512:j * 512 + 256],
                                          xT[k * 128:(k + 1) * 128, 0:256])
                        nc.sync.dma_start(t[:, j * 512 + 256:(j + 1) * 512],
                                          xT[k * 128:(k + 1) * 128, 256:512])
                else:
                    for j in range(SLAB):
                        k = s * SLAB + j
                        nc.gpsimd.dma_start(t[:, j * 512:(j + 1) * 512],
                                            xT[k * 128:(k + 1) * 128, n * 512:(n + 1) * 512])
                xsl_tiles[(n, s)] = t

            def nxt(n, s, ahead):
                i = n * NSLAB + s + ahead
                return i // NSLAB, i % NSLAB

            prefetch_x(0, 0)
            load_w_slab(0)
            prefetch_x(0, 1)
            load_w_slab(1)
            # constants + small SP-issued loads after the critical first slabs
            make_identity(nc, ident_f[:])
            nc.vector.tensor_copy(ident[:], ident_f[:])
            nc.vector.memset(ones_f[:], 1.0)
            nc.vector.tensor_copy(ones[:], ones_f[:])
            for r in range(4):
                nc.sync.dma_start(msk[:, r * 512:(r + 1) * 512],
                                  masks[r * 128:(r + 1) * 128, :])
            for c in range(4):
                nc.sync.dma_start(tabA[:, c * 512:(c + 1) * 512], ropeA[:, c * 512:(c + 1) * 512])
            for c in range(4):
                nc.sync.dma_start(tabB[:, c * 512:(c + 1) * 512], ropeB[:, c * 512:(c + 1) * 512])

            pending_vt = None

            def flush_vt():
                nonlocal pending_vt
                if pending_vt is None:
                    return
                kv_p, n_p = pending_vt
                b_p = n_p // 4
                ptr = T2([128, 512], "ptr", bf16)
                for i in range(4):
                    nc.tensor.transpose(ptr[:, i * 128:(i + 1) * 128],
                                        kv_p[:, 512 + i * 128: 512 + (i + 1) * 128],
                                        ident[:])
                c0 = 512 * (n_p % 4)
                nc.scalar.copy(Vb[b_p][:, c0:c0 + 512], ptr[:])
                pending_vt = None

            for n in range(NT // 512):
                b, tloc = n // 4, 512 * (n % 4)
                ps01 = T1([128, 1024], "ps01")
                ps23 = T2([128, 1024], "ps23")
                pskv = T1([128, 1024], "pskv")
                pst = [ps01, ps23, pskv]
                for s in range(NSLAB):
                    if n == 0 and s + 2 < NSLAB:
                        load_w_slab(s + 2)
                    prefetch_x(*nxt(n, s, 2))
                    xsl = xsl_tiles.pop((n, s))
                    for m in range(6):
                        dst = pst[m // 2][:, (m % 2) * 512:(m % 2) * 512 + 512]
                        for j in range(SLAB):
                            k = s * SLAB + j
                            nc.tensor.matmul(dst, stationary(m, k),
                                             xsl[:, j * 512:(j + 1) * 512],
                                             start=(k == 0), stop=(k == KC - 1))
                    if s == 0:
                        flush_vt()   # prev n-tile's V transposes, PE already warm
                # evict (ACT): bf16 staging. On the last n-tile, interleave
                # phase B's first score/exp stages between the evicts so the
                # B pipeline is hot the moment A's matmuls end.
                lastn = (n == NT // 512 - 1)
                kv = evp.tile([128, 1024], bf16, tag="kv", bufs=2, name="kv")
                nc.scalar.copy(kv[:], pskv[:])
                if lastn:
                    se_q.append(emit_S(items[0]))
                    se_q.append(emit_S(items[1]))
                qe23 = evp.tile([128, 1024], bf16, tag="qe23", bufs=2, name="qe23")
                nc.scalar.copy(qe23[:], ps23[:])
                if lastn:
                    se_q.append(emit_S(items[2]))
                qe01 = evp.tile([128, 1024], bf16, tag="qe01", bufs=2, name="qe01")
                nc.scalar.copy(qe01[:], ps01[:])
                # rope chains: 4 Q heads + K
                tA = tabA[:, tloc:tloc + 512]
                tB = tabB[:, tloc:tloc + 512]

                def rope(src, dst, eng):
                    sw = evp.tile([128, 512], f32, tag="sw", bufs=2, name="sw")
                    eng.tensor_copy(sw[0:64, :], src[64:128, :])
                    eng.tensor_copy(sw[64:128, :], src[0:64, :])
                    tt = evp.tile([128, 512], f32, tag="tt", bufs=2, name="tt")
                    eng.tensor_mul(tt[:], src, tA)
                    mm = evp.tile([128, 512], f32, tag="mm", bufs=2, name="mm")
                    eng.tensor_mul(mm[:], sw[:], tB)
                    eng.tensor_add(dst, tt[:], mm[:])

                # K first so KTb completes early; last n-tile's Q ropes go to
                # GPSIMD so the DVE queue is clear when phase B's masks arrive
                rope(kv[:, 0:512], KTb[b][:, tloc:tloc + 512], nc.vector)
                qeng = nc.gpsimd if n == NT // 512 - 1 else nc.vector
                for hh in range(HPC):
                    src = (qe01 if hh < 2 else qe23)[:, (hh % 2) * 512:(hh % 2) * 512 + 512]
                    rope(src, Qres[:, hh * NT + n * 512: hh * NT + (n + 1) * 512],
                         qeng)
                pending_vt = (kv, n)
            flush_vt()

        # ---------------- Phase B: attention -----------------
        with ExitStack() as sb:
            wop = sb.enter_context(tc.tile_pool(name="wop", bufs=1))
            wo_sb = wop.tile([128, HPC * D], bf16, tag="wo")
            for h in range(HPC):
                nc.sync.dma_start(wo_sb[:, h * D:(h + 1) * D], wo[h * 128:(h + 1) * 128, :])

            qstate = {}

            for i, it in enumerate(items):
                b, h, qt, p, idx, last = it
                se, ss = se_q.pop(0)
                if idx == 0:
                    qstate[(b, h, qt)] = T2([128, 1024], "ctxsm")
                cs = qstate[(b, h, qt)]
                ps_ctx = cs[:, 0:512]
                ps_sm = cs[:, 512:1024]
                for half in range(2):
                    kt = 2 * p + half
                    nc.tensor.matmul(ps_ctx, Vb[b][:, kt * 128:(kt + 1) * 128],
                                     se[:, half * 512:(half + 1) * 512],
                                     start=(idx == 0 and half == 0),
                                     stop=(last and half == 1))
                nc.tensor.matmul(ps_sm, ones[:], ss[:],
                                 start=(idx == 0), stop=last)
                if last:
                    rs = bp.tile([128, 512], f32, tag="rs", bufs=2, name="rs")
                    nc.vector.reciprocal_approx_fast(rs[:], ps_sm)
                    nc.vector.tensor_mul(
                        ctxr[:, h * NT + b * T + qt * 512: h * NT + b * T + (qt + 1) * 512],
                        ps_ctx, rs[:])
                    del qstate[(b, h, qt)]
                if i + LOOK < len(items):
                    se_q.append(emit_S(items[i + LOOK]))

            # ---------------- Phase C: output projection -----------------
            with ExitStack() as sc:
                obp = sc.enter_context(tc.tile_pool(name="obp", bufs=1))
                ii = 0
                for m in range(NT // 128):
                    for n2 in range(D // 1024):
                        pso = (T1 if ii % 2 == 0 else T2)([128, 1024], "pso")
                        ii += 1
                        for h in range(HPC):
                            stat = ctxr[:, h * NT + m * 128: h * NT + (m + 1) * 128]
                            for half in range(2):
                                col = n2 * 1024 + half * 512
                                nc.tensor.matmul(pso[:, half * 512:(half + 1) * 512],
                                                 stat,
                                                 wo_sb[:, h * D + col: h * D + col + 512],
                                                 start=(h == 0), stop=(h == HPC - 1))
                        ob = obp.tile([128, 1024], bf16, tag="ob", bufs=4, name="ob")
                        nc.scalar.copy(ob[:], pso[:])
                        nc.sync.dma_start(outp[m * 128:(m + 1) * 128, n2 * 1024:(n2 + 1) * 1024],
                                          ob[:])

    nc.compile()
    return nc


def _get_nc():
    if "nc" not in _NC_CACHE:
        _NC_CACHE["nc"] = _build_program()
    return _NC_CACHE["nc"]


def _rope_tables():
    j = np.arange(0, DH, 2, dtype=np.float32) / np.float32(DH)
    inv_freq = (np.float32(1.0) / (np.float32(ROPE_BASE) ** j)).astype(np.float32)
    t = np.arange(T, dtype=np.float32)
    freqs = np.outer(t, inv_freq).astype(np.float32)   # (T, 64)
    c = np.cos(freqs).astype(np.float32).T             # (64, T)
    s = np.sin(freqs).astype(np.float32).T
    A = np.vstack([c, c]).astype(np.float32)           # (128, T)
    Bt = np.vstack([-s, s]).astype(np.float32)
    return np.ascontiguousarray(A), np.ascontiguousarray(Bt)


def _causal_masks():
    m = np.zeros((512, 512), dtype=np.float32)
    for r in range(4):
        p = np.arange(128)[:, None]
        f = np.arange(512)[None, :]
        m[r * 128:(r + 1) * 128, :] = (r * 128 + p <= f).astype(np.float32)
    return m


def _make_in_maps(x, Wq, Wk, Wv, Wo):
    xTb = np.ascontiguousarray(
        np.asarray(x, dtype=np.float32).reshape(NT, D).T).astype(BF)
    A, Bt = _rope_tables()
    mskh = _causal_masks().astype(BF)
    Wq = np.asarray(Wq, dtype=np.float32)
    Wk = np.asarray(Wk, dtype=np.float32)
    Wv = np.asarray(Wv, dtype=np.float32)
    Wo = np.asarray(Wo, dtype=np.float32)
    in_maps = []
    for g in range(8):
        in_maps.append({
            "xT": xTb,
            "wq": np.ascontiguousarray(Wq[:, g * FPC:(g + 1) * FPC]).astype(BF),
            "wk": np.ascontiguousarray(Wk[:, g * DH:(g + 1) * DH]).astype(BF),
            "wv": np.ascontiguousarray(Wv[:, g * DH:(g + 1) * DH]).astype(BF),
            "wo": np.ascontiguousarray(Wo[g * FPC:(g + 1) * FPC, :]).astype(BF),
            "ropeA": A,
            "ropeB": Bt,
            "masks": mskh,
        })
    return in_maps


def kernel(x, Wq, Wk, Wv, Wo):
    nc = _get_nc()
    in_maps = _make_in_maps(x, Wq, Wk, Wv, Wo)
    res = run_bass_kernel_spmd(nc, in_maps, list(range(8)))
    acc = res.results[0]["outp"].astype(np.float32)
    for g in range(1, 8):
        acc = acc + res.results[g]["outp"].astype(np.float32)
    return np.ascontiguousarray(acc.reshape(B, T, D), dtype=np.float32)


# revision 22
# speedup vs baseline: 1.0160x; 1.0160x over previous
"""GroupedQueryAttention TRN2 Bass kernel, sharded over 8 NeuronCores.

Problem (hardcoded): B=2, T=2048, D=4096, 32 Q heads x 128, 8 KV groups x 128,
RoPE (base 5e5), causal, out = ctx @ Wo.

Sharding: core g owns Q heads 4g..4g+3 (Wq columns 512g:512g+512), KV group g
(Wk/Wv columns 128g:128g+128), and Wo rows 512g:512g+512 (row-parallel).
Each core computes a full-shape partial output; host sums the 8 partials.

All matmuls run in bf16 (host-converted inputs), accumulation in f32 PSUM.
Q and attention context stay resident in SBUF (no DRAM roundtrips).
One PSUM pool spans all three phases (no pool-transition barriers); softmax
normalization runs off the PE (fast DVE reciprocal + GPSIMD partition
broadcast); weight/x loads issue from the GPSIMD sequencer with merged
access patterns.
"""
import sys
import numpy as np

for _p in ("/opt/trn_rl_repo", "/root/.axon_site", "/root/.axon_site/_ro/trn_rl_repo"):
    if _p not in sys.path:
        sys.path.append(_p)

from contextlib import ExitStack

import ml_dtypes

import concourse.bass as bass
import concourse.tile as tile
from concourse import bacc, mybir
from concourse.bass_utils import run_bass_kernel_spmd
from concourse.masks import make_identity

B, T, D = 2, 2048, 4096
NH, NKV, DH = 32, 8, 128
HPC = NH // 8          # 4 q heads per core
FPC = HPC * DH         # 512 q features per core
ROPE_BASE = 500000.0
NT = B * T             # 4096 tokens
KC = D // 128          # 32 contraction chunks
SLAB = 4
NSLAB = KC // SLAB
f32 = mybir.dt.float32
bf16 = mybir.dt.bfloat16
EXP_SCALE = 1.0 / float(np.sqrt(DH))
BF = ml_dtypes.bfloat16

_NC_CACHE = {}


def _build_program():
    nc = bacc.Bacc("TRN2", target_bir_lowering=False, debug=False)

    xT = nc.dram_tensor("xT", [D, NT], bf16, kind="ExternalInput").ap()
    wq = nc.dram_tensor("wq", [D, FPC], bf16, kind="ExternalInput").ap()
    wk = nc.dram_tensor("wk", [D, DH], bf16, kind="ExternalInput").ap()
    wv = nc.dram_tensor("wv", [D, DH], bf16, kind="ExternalInput").ap()
    wo = nc.dram_tensor("wo", [FPC, D], bf16, kind="ExternalInput").ap()
    ropeA = nc.dram_tensor("ropeA", [128, T], f32, kind="ExternalInput").ap()
    ropeB = nc.dram_tensor("ropeB", [128, T], f32, kind="ExternalInput").ap()
    masks = nc.dram_tensor("masks", [512, 512], bf16, kind="ExternalInput").ap()
    outp = nc.dram_tensor("outp", [NT, D], bf16, kind="ExternalOutput").ap()

    with tile.TileContext(nc) as tc, ExitStack() as s0:
        top = s0.enter_context(tc.tile_pool(name="top", bufs=1))
        KTb = [top.tile([128, T], bf16, tag=f"KT{i}", name=f"KT{i}") for i in range(B)]
        Vb = [top.tile([128, T], bf16, tag=f"V{i}", name=f"V{i}") for i in range(B)]
        Qres = top.tile([128, HPC * NT], bf16, tag="qres")
        ctxr = top.tile([128, HPC * NT], bf16, tag="ctxr")
        msk = top.tile([128, 4 * 512], bf16, tag="msk")
        ident = top.tile([128, 128], bf16, tag="ident")
        ident_f = top.tile([128, 128], f32, tag="ident_f")
        ones_f = top.tile([128, 128], f32, tag="ones_f")
        ones = top.tile([128, 128], bf16, tag="ones")
        # phase-B SBUF tiles live at top level so B's pipeline never waits on
        # phase-A pool release
        bp = s0.enter_context(tc.tile_pool(name="bp", bufs=1))
        # single PSUM pool for all phases: two ping-pong slot groups
        psp = s0.enter_context(tc.tile_pool(name="psp", bufs=1, space="PSUM"))

        def T1(shape, name, dt=f32):
            return psp.tile(shape, dt, tag="T1", bufs=2, name=name)

        def T2(shape, name, dt=f32):
            return psp.tile(shape, dt, tag="T2", bufs=2, name=name)

        # phase-B item stream (defined early: the tail of phase A primes it)
        items = []
        for b_ in range(B):
            for h_ in range(HPC):
                for qt_ in range(4):
                    plist = [2 * qt_, 2 * qt_ + 1] + list(range(0, 2 * qt_))
                    for idx_, p_ in enumerate(plist):
                        items.append((b_, h_, qt_, p_, idx_, idx_ == len(plist) - 1))
        LOOK = 4
        se_q = []

        def emit_S(it):
            b, h, qt, p, idx, last = it
            st = T1([128, 1024], "st")
            qmv = Qres[:, h * NT + b * T + qt * 512: h * NT + b * T + (qt + 1) * 512]
            for half in range(2):
                kt = 2 * p + half
                nc.tensor.matmul(st[:, half * 512:(half + 1) * 512],
                                 KTb[b][:, kt * 128:(kt + 1) * 128],
                                 qmv, start=True, stop=True)
            se = bp.tile([128, 1024], bf16, tag="se", bufs=6, name="se")
            nc.scalar.activation(se[:], st[:],
                                 mybir.ActivationFunctionType.Exp,
                                 scale=EXP_SCALE)
            if p >= 2 * qt:
                r = (p - 2 * qt) * 2
                nc.vector.tensor_mul(se[:], se[:], msk[:, r * 512:(r + 2) * 512])
            # pair-sum on DVE so one sum-MM per pair suffices
            ss = bp.tile([128, 512], bf16, tag="ss", bufs=6, name="ss")
            nc.vector.tensor_add(ss[:], se[:, 0:512], se[:, 512:1024])
            return se, ss

        # ---------------- Phase A: projections + RoPE -----------------
        with ExitStack() as sa:
            wp = sa.enter_context(tc.tile_pool(name="wts", bufs=1))
            wq_sb = wp.tile([128, KC * FPC], bf16, tag="wq")
            wk_sb = wp.tile([128, KC * DH], bf16, tag="wk")
            wv_sb = wp.tile([128, KC * DH], bf16, tag="wv")
            tabA = wp.tile([128, T], f32, tag="tabA")
            tabB = wp.tile([128, T], f32, tag="tabB")

            # x/weight loads issue from the GPSIMD sequencer (cheap SWDGE
            # dispatch) with slab-merged access patterns (few large starts)
            def load_w_slab(s):
                r0 = s * 512
                if s == 0:
                    # fine-grained wq for first-matmul latency
                    nc.gpsimd.dma_start(wq_sb[:, 0:256], wq[0:128, 0:256])
                    nc.gpsimd.dma_start(wq_sb[:, 256:512], wq[0:128, 256:512])
                    for k in range(1, SLAB):
                        nc.gpsimd.dma_start(wq_sb[:, k * FPC:(k + 1) * FPC],
                                            wq[k * 128:(k + 1) * 128, :])
                    nc.gpsimd.dma_start(
                        wk_sb[:, 0:SLAB * DH].rearrange("p (k f) -> p k f", k=4),
                        wk[0:512, :].rearrange("(k p) f -> p k f", k=4))
                    nc.gpsimd.dma_start(
                        wv_sb[:, 0:SLAB * DH].rearrange("p (k f) -> p k f", k=4),
                        wv[0:512, :].rearrange("(k p) f -> p k f", k=4))
                    return
                for half in range(2):
                    src = wq[r0 + half * 256:r0 + (half + 1) * 256, :].rearrange(
                        "(k p) f -> p k f", k=2)
                    dst = wq_sb[:, (s * SLAB + half * 2) * FPC:
                                (s * SLAB + (half + 1) * 2) * FPC].rearrange(
                        "p (k f) -> p k f", k=2)
                    nc.gpsimd.dma_start(dst, src)
                nc.gpsimd.dma_start(
                    wk_sb[:, s * SLAB * DH:(s + 1) * SLAB * DH].rearrange(
                        "p (k f) -> p k f", k=4),
                    wk[r0:r0 + 512, :].rearrange("(k p) f -> p k f", k=4))
                nc.gpsimd.dma_start(
                    wv_sb[:, s * SLAB * DH:(s + 1) * SLAB * DH].rearrange(
                        "p (k f) -> p k f", k=4),
                    wv[r0:r0 + 512, :].rearrange("(k p) f -> p k f", k=4))

            xsp = sa.enter_context(tc.tile_pool(name="xs", bufs=1))
            evp = sa.enter_context(tc.tile_pool(name="ev", bufs=1))

            def stationary(m, k):
                if m < HPC:
                    return wq_sb[:, k * FPC + m * 128: k * FPC + (m + 1) * 128]
                if m == HPC:
                    return wk_sb[:, k * DH:(k + 1) * DH]
                return wv_sb[:, k * DH:(k + 1) * DH]

            xsl_tiles = {}

            def prefetch_x(n, s):
                if n >= NT // 512:
                    return
                t = xsp.tile([128, SLAB * 512], bf16, tag="xs", bufs=3, name="xsl")
                if n == 0 and s < 2:
                    for j in range(SLAB):
                        k = s * SLAB + j
                        nc.sync.dma_start(t[:, j * 512:j * 512 + 256],
                                          xT[k * 128:(k + 1) * 128, 0:256])
                        nc.sync.dma_start(t[:, j * 512 + 256:(j + 1) * 512],
                                          xT[k * 128:(k + 1) * 128, 256:512])
                else:
                    for j in range(SLAB):
                        k = s * SLAB + j
                        nc.gpsimd.dma_start(t[:, j * 512:(j + 1) * 512],
                                            xT[k * 128:(k + 1) * 128, n * 512:(n + 1) * 512])
                xsl_tiles[(n, s)] = t

            def nxt(n, s, ahead):
                i = n * NSLAB + s + ahead
                return i // NSLAB, i % NSLAB

            prefetch_x(0, 0)
            load_w_slab(0)
            prefetch_x(0, 1)
            load_w_slab(1)
            # constants + small SP-issued loads after the critical first slabs
            make_identity(nc, ident_f[:])
            nc.vector.tensor_copy(ident[:], ident_f[:])
            nc.vector.memset(ones_f[:], 1.0)
            nc.vector.tensor_copy(ones[:], ones_f[:])
            for r in range(4):
                nc.sync.dma_start(msk[:, r * 512:(r + 1) * 512],
                                  masks[r * 128:(r + 1) * 128, :])
            for c in range(4):
                nc.sync.dma_start(tabA[:, c * 512:(c + 1) * 512], ropeA[:, c * 512:(c + 1) * 512])
            for c in range(4):
                nc.sync.dma_start(tabB[:, c * 512:(c + 1) * 512], ropeB[:, c * 512:(c + 1) * 512])

            pending_vt = None

            def flush_vt():
                nonlocal pending_vt
                if pending_vt is None:
                    return
                kv_p, n_p = pending_vt
                b_p = n_p // 4
                ptr = T2([128, 512], "ptr", bf16)
                for i in range(4):
                    nc.tensor.transpose(ptr[:, i * 128:(i + 1) * 128],
                                        kv_p[:, 512 + i * 128: 512 + (i + 1) * 128],
                                        ident[:])
                c0 = 512 * (n_p % 4)
                nc.scalar.copy(Vb[b_p][:, c0:c0 + 512], ptr[:])
                pending_vt = None

            for n in range(NT // 512):
                b, tloc = n // 4, 512 * (n % 4)
                ps01 = T1([128, 1024], "ps01")
                ps23 = T2([128, 1024], "ps23")
                pskv = T1([128, 1024], "pskv")
                pst = [ps01, ps23, pskv]
                for s in range(NSLAB):
                    if n == 0 and s + 2 < NSLAB:
                        load_w_slab(s + 2)
                    prefetch_x(*nxt(n, s, 2))
                    xsl = xsl_tiles.pop((n, s))
                    for m in range(6):
                        dst = pst[m // 2][:, (m % 2) * 512:(m % 2) * 512 + 512]
                        for j in range(SLAB):
                            k = s * SLAB + j
                            nc.tensor.matmul(dst, stationary(m, k),
                                             xsl[:, j * 512:(j + 1) * 512],
                                             start=(k == 0), stop=(k == KC - 1))
                    if s == 0:
                        flush_vt()   # prev n-tile's V transposes, PE already warm
                # evict (ACT): bf16 staging. On the last n-tile, interleave
                # phase B's first score/exp stages between the evicts so the
                # B pipeline is hot the moment A's matmuls end.
                lastn = (n == NT // 512 - 1)
                kv = evp.tile([128, 1024], bf16, tag="kv", bufs=2, name="kv")
                nc.scalar.copy(kv[:], pskv[:])
                if lastn:
                    se_q.append(emit_S(items[0]))
                    se_q.append(emit_S(items[1]))
                qe23 = evp.tile([128, 1024], bf16, tag="qe23", bufs=2, name="qe23")
                nc.scalar.copy(qe23[:], ps23[:])
                if lastn:
                    se_q.append(emit_S(items[2]))
                qe01 = evp.tile([128, 1024], bf16, tag="qe01", bufs=2, name="qe01")
                nc.scalar.copy(qe01[:], ps01[:])
                if lastn:
                    se_q.append(emit_S(items[3]))
                # rope chains: 4 Q heads + K
                tA = tabA[:, tloc:tloc + 512]
                tB = tabB[:, tloc:tloc + 512]

                def rope(src, dst, eng):
                    sw = evp.tile([128, 512], f32, tag="sw", bufs=2, name="sw")
                    eng.tensor_copy(sw[0:64, :], src[64:128, :])
                    eng.tensor_copy(sw[64:128, :], src[0:64, :])
                    tt = evp.tile([128, 512], f32, tag="tt", bufs=2, name="tt")
                    eng.tensor_mul(tt[:], src, tA)
                    mm = evp.tile([128, 512], f32, tag="mm", bufs=2, name="mm")
                    eng.tensor_mul(mm[:], sw[:], tB)
                    eng.tensor_add(dst, tt[:], mm[:])

                # K first so KTb completes early; last n-tile's Q ropes go to
                # GPSIMD so the DVE queue is clear when phase B's masks arrive
                rope(kv[:, 0:512], KTb[b][:, tloc:tloc + 512], nc.vector)
                qeng = nc.gpsimd if n == NT // 512 - 1 else nc.vector
                for hh in range(HPC):
                    src = (qe01 if hh < 2 else qe23)[:, (hh % 2) * 512:(hh % 2) * 512 + 512]
                    rope(src, Qres[:, hh * NT + n * 512: hh * NT + (n + 1) * 512],
                         qeng)
                pending_vt = (kv, n)
            flush_vt()

        # ---------------- Phase B: attention -----------------
        with ExitStack() as sb:
            wop = sb.enter_context(tc.tile_pool(name="wop", bufs=1))
            wo_sb = wop.tile([128, HPC * D], bf16, tag="wo")
            for h in range(HPC):
                nc.sync.dma_start(wo_sb[:, h * D:(h + 1) * D], wo[h * 128:(h + 1) * 128, :])

            qstate = {}

            for i, it in enumerate(items):
                b, h, qt, p, idx, last = it
                se, ss = se_q.pop(0)
                if idx == 0:
                    qstate[(b, h, qt)] = T2([128, 1024], "ctxsm")
                cs = qstate[(b, h, qt)]
                ps_ctx = cs[:, 0:512]
                ps_sm = cs[:, 512:1024]
                for half in range(2):
                    kt = 2 * p + half
                    nc.tensor.matmul(ps_ctx, Vb[b][:, kt * 128:(kt + 1) * 128],
                                     se[:, half * 512:(half + 1) * 512],
                                     start=(idx == 0 and half == 0),
                                     stop=(last and half == 1))
                nc.tensor.matmul(ps_sm, ones[:], ss[:],
                                 start=(idx == 0), stop=last)
                if last:
                    rs = bp.tile([128, 512], f32, tag="rs", bufs=2, name="rs")
                    nc.vector.reciprocal_approx_fast(rs[:], ps_sm)
                    nc.vector.tensor_mul(
                        ctxr[:, h * NT + b * T + qt * 512: h * NT + b * T + (qt + 1) * 512],
                        ps_ctx, rs[:])
                    del qstate[(b, h, qt)]
                if i + LOOK < len(items):
                    se_q.append(emit_S(items[i + LOOK]))

            # ---------------- Phase C: output projection -----------------
            with ExitStack() as sc:
                obp = sc.enter_context(tc.tile_pool(name="obp", bufs=1))
                ii = 0
                for m in range(NT // 128):
                    for n2 in range(D // 1024):
                        pso = (T1 if ii % 2 == 0 else T2)([128, 1024], "pso")
                        ii += 1
                        for h in range(HPC):
                            stat = ctxr[:, h * NT + m * 128: h * NT + (m + 1) * 128]
                            for half in range(2):
                                col = n2 * 1024 + half * 512
                                nc.tensor.matmul(pso[:, half * 512:(half + 1) * 512],
                                                 stat,
                                                 wo_sb[:, h * D + col: h * D + col + 512],
                                                 start=(h == 0), stop=(h == HPC - 1))
                        ob = obp.tile([128, 1024], bf16, tag="ob", bufs=4, name="ob")
                        nc.scalar.copy(ob[:], pso[:])
                        nc.sync.dma_start(outp[m * 128:(m + 1) * 128, n2 * 1024:(n2 + 1) * 1024],
                                          ob[:])

    nc.compile()
    return nc


def _get_nc():
    if "nc" not in _NC_CACHE:
        _NC_CACHE["nc"] = _build_program()
    return _NC_CACHE["nc"]


def _rope_tables():
    j = np.arange(0, DH, 2, dtype=np.float32) / np.float32(DH)
    inv_freq = (np.float32(1.0) / (np.float32(ROPE_BASE) ** j)).astype(np.float32)
    t = np.arange(T, dtype=np.float32)
    freqs = np.outer(t, inv_freq).astype(np.float32)   # (T, 64)
    c = np.cos(freqs).astype(np.float32).T             # (64, T)
    s = np.sin(freqs).astype(np.float32).T
    A = np.vstack([c, c]).astype(np.float32)           # (128, T)
    Bt = np.vstack([-s, s]).astype(np.float32)
    return np.ascontiguousarray(A), np.ascontiguousarray(Bt)


def _causal_masks():
    m = np.zeros((512, 512), dtype=np.float32)
    for r in range(4):
        p = np.arange(128)[:, None]
        f = np.arange(512)[None, :]
        m[r * 128:(r + 1) * 128, :] = (r * 128 + p <= f).astype(np.float32)
    return m


def _make_in_maps(x, Wq, Wk, Wv, Wo):
    xTb = np.ascontiguousarray(
        np.asarray(x, dtype=np.float32).reshape(NT, D).T).astype(BF)
    A, Bt = _rope_tables()
    mskh = _causal_masks().astype(BF)
    Wq = np.asarray(Wq, dtype=np.float32)
    Wk = np.asarray(Wk, dtype=np.float32)
    Wv = np.asarray(Wv, dtype=np.float32)
    Wo = np.asarray(Wo, dtype=np.float32)
    in_maps = []
    for g in range(8):
        in_maps.append({
            "xT": xTb,
            "wq": np.ascontiguousarray(Wq[:, g * FPC:(g + 1) * FPC]).astype(BF),
            "wk": np.ascontiguousarray(Wk[:, g * DH:(g + 1) * DH]).astype(BF),
            "wv": np.ascontiguousarray(Wv[:, g * DH:(g + 1) * DH]).astype(BF),
            "wo": np.ascontiguousarray(Wo[g * FPC:(g + 1) * FPC, :]).astype(BF),
            "ropeA": A,
            "ropeB": Bt,
            "masks": mskh,
        })
    return in_maps


def kernel(x, Wq, Wk, Wv, Wo):
    nc = _get_nc()
    in_maps = _make_in_maps(x, Wq, Wk, Wv, Wo)
    res = run_bass_kernel_spmd(nc, in_maps, list(range(8)))
    acc = res.results[0]["outp"].astype(np.float32)
    for g in range(1, 8):
        acc = acc + res.results[g]["outp"].astype(np.float32)
    return np.ascontiguousarray(acc.reshape(B, T, D), dtype=np.float32)
